# revision 6
# baseline (speedup 1.0000x reference)
"""Trainium2 Bass kernel for nn_Dense_25202868092933.

Computes: outs = einsum('bit,oi->bot', inputs, W); then a 64-step
leaky-integrate-and-fire neuron scan with hard reset:
    mem = mult*mem + scale*outs[..., t];  spk = (mem <= -0.5);  mem *= (1-spk)

Strategy (8 NeuronCores, no cross-core communication):
  - Tensor-parallel over out_features: each core owns 512 rows of W,
    the scale/mult vectors for those rows, and the scan state.
  - scale is folded into W on the host (W' = diag(scale) @ W), so the
    matmul directly produces scale*outs.
  - The matmul runs as a 3-pass bf16 hi/lo split (x_hi@W_hi + x_lo@W_hi
    + x_hi@W_lo), which reproduces fp32 accuracy to ~1.5e-6 abs on the
    pre-scan activations (measured: zero spike flips) at bf16 PE rate.
  - bt = t*64+b is the matmul moving dimension; blocks of 256 bt (4
    timesteps) pipeline matmul (PE) -> psum copy (ACT) -> scan (DVE)
    -> spike DMA, so the scan hides under the next block's matmuls.
  - The kernel emits "no-spike" indicators (mem > thr); the host
    computes spikes = 1 - nspk (exact complement of float 0/1).
"""

import os
import sys

for _p in ("/opt/trn_rl_repo", "/root/.axon_site", "/root/.axon_site/_ro/trn_rl_repo",
           "/root/.axon_site/_ro/pypackages"):
    if os.path.isdir(_p) and _p not in sys.path:
        sys.path.append(_p)

from contextlib import ExitStack

import ml_dtypes
import numpy as np

import concourse.bass as bass  # noqa: F401  (registers engine classes)
import concourse.mybir as mybir
import concourse.tile as tile
from concourse import bacc
from concourse.bass_utils import run_bass_kernel_spmd

# Problem constants
B, F_IN, F_OUT, T = 64, 4096, 4096, 64
KVCO = 5.0e8
KPD = 1.0 / np.pi
TIMESTEP = 1e-9
SCALING = KVCO * KPD * TIMESTEP / 2.0
THRESHOLD = -0.5

NCORES = 8
FS = F_OUT // NCORES      # 512 out-features per core
P = 128                   # partitions
M = FS // P               # 4 feature tiles per core
KO = F_IN // P            # 32 contraction tiles
BT = B * T                # 4096 moving dim (t-major: bt = t*B + b)
NJ = 256                  # bt per block
NB = BT // NJ             # 16 blocks
TL = NJ // B              # 4 timesteps per block

_dt = mybir.dt

_CACHE = {}


def _build_program():
    """Emit the SPMD Tile program (identical on all 8 cores)."""
    nc = bacc.Bacc("TRN2", target_bir_lowering=False, debug=False)

    xhi_d = nc.dram_tensor("xhi", [NB, P, KO * NJ], _dt.bfloat16, kind="ExternalInput").ap()
    xlo_d = nc.dram_tensor("xlo", [NB, P, KO * NJ], _dt.bfloat16, kind="ExternalInput").ap()
    whi_d = nc.dram_tensor("whi", [P, M * KO * P], _dt.bfloat16, kind="ExternalInput").ap()
    wlo_d = nc.dram_tensor("wlo", [P, M * KO * P], _dt.bfloat16, kind="ExternalInput").ap()
    mlt_d = nc.dram_tensor("multf", [P, M * B], _dt.float32, kind="ExternalInput").ap()
    out_d = nc.dram_tensor("nspk", [P, M * NB * NJ], _dt.float32, kind="ExternalOutput").ap()
    out4 = out_d.rearrange("p (m nb j) -> p m nb j", m=M, nb=NB)

    with tile.TileContext(nc) as tc, ExitStack() as ctx:
        wpool = ctx.enter_context(tc.tile_pool(name="wpool", bufs=1))
        cpool = ctx.enter_context(tc.tile_pool(name="cpool", bufs=1))
        xpool = ctx.enter_context(tc.tile_pool(name="xpool", bufs=2))
        ypool = ctx.enter_context(tc.tile_pool(name="ypool", bufs=2))
        npool = ctx.enter_context(tc.tile_pool(name="npool", bufs=2))
        spool = ctx.enter_context(tc.tile_pool(name="spool", bufs=1))
        tpool = ctx.enter_context(tc.tile_pool(name="tpool", bufs=2))
        pspool = ctx.enter_context(tc.tile_pool(name="pspool", bufs=8, space="PSUM"))

        whi4 = whi_d.rearrange("p (m k c) -> p m k c", m=M, k=KO)
        wlo4 = wlo_d.rearrange("p (m k c) -> p m k c", m=M, k=KO)
        # Startup-latency-critical ordering: first matmul needs w_hi[m=0]
        # and the first half of x block 0. Interleave across the two DMA
        # paths (sync=HWDGE, gpsimd=SWDGE) in ~1 MB chunks so the PE can
        # start ~6 us in instead of waiting ~40 us for serialized loads.
        # Three parallel DMA queues at startup, ordered by first use:
        #   sync:   w_hi[0..3], then steady-state xh halves
        #   gpsimd: xl0 first half, xh0 second half, then steady xl + outs
        #   scalar: xh0 first half, w_lo[0..3], xl0 second half
        w_hi = wpool.tile([P, M, KO, P], _dt.bfloat16, tag="whi", name="w_hi")
        w_lo = wpool.tile([P, M, KO, P], _dt.bfloat16, tag="wlo", name="w_lo")
        xh0 = xpool.tile([P, KO, NJ], _dt.bfloat16, tag="xh", name="xh0")
        xl0 = xpool.tile([P, KO, NJ], _dt.bfloat16, tag="xl", name="xl0")
        xh0src = xhi_d[0].rearrange("p (k j) -> p k j", k=KO)
        xl0src = xlo_d[0].rearrange("p (k j) -> p k j", k=KO)
        H = KO // 2
        nc.sync.dma_start(w_hi[:, 0], whi4[:, 0])
        nc.scalar.dma_start(xh0[:, 0:H, :], xh0src[:, 0:H, :])
        nc.gpsimd.dma_start(xl0[:, 0:H, :], xl0src[:, 0:H, :])
        nc.scalar.dma_start(w_lo[:, 0], wlo4[:, 0])
        nc.gpsimd.dma_start(xh0[:, H:, :], xh0src[:, H:, :])
        nc.scalar.dma_start(xl0[:, H:, :], xl0src[:, H:, :])
        for m in range(1, M):
            nc.sync.dma_start(w_hi[:, m], whi4[:, m])
            nc.scalar.dma_start(w_lo[:, m], wlo4[:, m])

        mlt = cpool.tile([P, M, B], _dt.float32, tag="mlt", name="mlt")
        nc.gpsimd.dma_start(mlt[:], mlt_d.rearrange("p (m b) -> p m b", m=M))

        mem = spool.tile([P, M, B], _dt.float32, tag="mem", name="mem")
        nc.vector.memset(mem[:], 0.0)

        for nb in range(NB):
            if nb == 0:
                xh, xl = xh0, xl0
            else:
                xh = xpool.tile([P, KO, NJ], _dt.bfloat16, tag="xh", name="xh")
                xl = xpool.tile([P, KO, NJ], _dt.bfloat16, tag="xl", name="xl")
                xsrc = xhi_d[nb].rearrange("p (k j) -> p k j", k=KO)
                lsrc = xlo_d[nb].rearrange("p (k j) -> p k j", k=KO)
                nc.sync.dma_start(xh[:, 0:KO // 2, :], xsrc[:, 0:KO // 2, :])
                nc.gpsimd.dma_start(xl[:, 0:KO // 2, :], lsrc[:, 0:KO // 2, :])
                nc.sync.dma_start(xh[:, KO // 2:, :], xsrc[:, KO // 2:, :])
                nc.gpsimd.dma_start(xl[:, KO // 2:, :], lsrc[:, KO // 2:, :])

            y = ypool.tile([P, M, TL, B], _dt.float32, tag="y", name="y")
            for m in range(M):
                ps = pspool.tile([P, TL, B], _dt.float32, tag="ps", name="ps")
                for ko in range(KO):
                    # weight-reuse-friendly order: whi used twice in a row
                    nc.tensor.matmul(ps[:], w_hi[:, m, ko, :], xh[:, ko, :],
                                     start=(ko == 0), stop=False)
                    nc.tensor.matmul(ps[:], w_hi[:, m, ko, :], xl[:, ko, :],
                                     start=False, stop=False)
                    nc.tensor.matmul(ps[:], w_lo[:, m, ko, :], xh[:, ko, :],
                                     start=False, stop=(ko == KO - 1))
                nc.scalar.copy(y[:, m, :, :], ps[:])

            nsp = npool.tile([P, M, TL, B], _dt.float32, tag="nsp", name="nsp")
            for tl in range(TL):
                tmp = tpool.tile([P, M, B], _dt.float32, tag="tmp", name="tmp")
                nc.vector.tensor_mul(tmp[:], mem[:], mlt[:])
                nc.vector.tensor_add(mem[:], tmp[:], y[:, :, tl, :])
                nc.vector.tensor_scalar(nsp[:, :, tl, :], mem[:], THRESHOLD, None,
                                        mybir.AluOpType.is_gt)
                nc.vector.tensor_mul(mem[:], mem[:], nsp[:, :, tl, :])

            nc.gpsimd.dma_start(out4[:, :, nb, :],
                                nsp[:].rearrange("p m tl b -> p m (tl b)"))

    nc.compile()
    return nc


def _prep_inputs(inputs: np.ndarray, W: np.ndarray):
    """Host-side preprocessing: neuron constants, scale folding, hi/lo
    bf16 split, and per-core data layouts."""
    f32 = np.float32
    rs = W.sum(axis=1, dtype=np.float64).astype(f32)
    ncst = np.maximum((f32(1.0) + rs) / f32(SCALING), f32(0.0)).astype(f32)
    scale = (f32(-1.0) / (f32(1.0) + ncst)).astype(f32)
    mult = ((ncst - f32(1.0)) / (ncst + f32(1.0))).astype(f32)

    Ws = (W * scale[:, None]).astype(f32)          # fold scale into W rows
    wT = np.ascontiguousarray(Ws.T)                # [F_IN, F_OUT]
    wT_hi = wT.astype(ml_dtypes.bfloat16)
    wT_lo = (wT - wT_hi.astype(f32)).astype(ml_dtypes.bfloat16)

    # x_flat[i, t*B + b] = inputs[b, i, t]
    x_flat = np.ascontiguousarray(inputs.transpose(1, 2, 0)).reshape(F_IN, BT)
    x_hi = x_flat.astype(ml_dtypes.bfloat16)
    x_lo = (x_flat - x_hi.astype(f32)).astype(ml_dtypes.bfloat16)

    def xblocks(xa):
        # [F_IN, BT] -> [NB, P, KO*NJ]; per-partition contiguous 16 KB
        return np.ascontiguousarray(
            xa.reshape(KO, P, NB, NJ).transpose(2, 1, 0, 3)).reshape(NB, P, KO * NJ)

    xh_b = xblocks(x_hi)
    xl_b = xblocks(x_lo)

    in_maps = []
    for c in range(NCORES):
        sl = slice(c * FS, (c + 1) * FS)

        def wlayout(wa):
            # [F_IN, FS] -> [P, M*KO*P]: w[p, m, ko, cc] = wT[ko*P+p, m*P+cc]
            return np.ascontiguousarray(
                wa[:, sl].reshape(KO, P, M, P).transpose(1, 2, 0, 3)).reshape(P, M * KO * P)

        mc = mult[sl].reshape(M, P).T              # [P, M]
        multf = np.ascontiguousarray(
            np.broadcast_to(mc[:, :, None], (P, M, B))).reshape(P, M * B).astype(f32)

        in_maps.append({
            "xhi": xh_b, "xlo": xl_b,
            "whi": wlayout(wT_hi), "wlo": wlayout(wT_lo),
            "multf": multf,
        })
    return in_maps


def kernel(inputs: np.ndarray, W: np.ndarray) -> np.ndarray:
    if "nc" not in _CACHE:
        _CACHE["nc"] = _build_program()
    nc = _CACHE["nc"]

    in_maps = _prep_inputs(np.asarray(inputs, np.float32), np.asarray(W, np.float32))

    kw = {}
    if os.environ.get("KERNEL_TRACE"):
        kw = {"trace": True}
    res = run_bass_kernel_spmd(nc, in_maps, core_ids=list(range(NCORES)), **kw)
    _CACHE["last_result"] = res
    if res.exec_time_ns is not None:
        print(f"HW exec time: {res.exec_time_ns} ns")

    full = np.empty((B, F_OUT, T), np.float32)
    for c in range(NCORES):
        a = res.results[c]["nspk"].reshape(P, M, NB, TL, B)
        a = a.transpose(4, 1, 0, 2, 3)             # [b, m, p, nb, tl]
        full[:, c * FS:(c + 1) * FS, :] = a.reshape(B, FS, T)
    return (np.float32(1.0) - full).astype(np.float32)


# revision 12
# speedup vs baseline: 1.0591x; 1.0591x over previous
"""Trainium2 Bass kernel for nn_Dense_25202868092933.

Computes: outs = einsum('bit,oi->bot', inputs, W); then a 64-step
leaky-integrate-and-fire neuron scan with hard reset:
    mem = mult*mem + scale*outs[..., t];  spk = (mem <= -0.5);  mem *= (1-spk)

Strategy (8 NeuronCores, no cross-core communication):
  - Tensor-parallel over out_features: each core owns 512 rows of W,
    the scale/mult vectors for those rows, and the scan state.
  - scale is folded into W on the host (W' = diag(scale) @ W), so the
    matmul directly produces scale*outs.
  - The matmul runs as a 3-pass bf16 hi/lo split (x_hi@W_hi + x_lo@W_hi
    + x_hi@W_lo), which reproduces fp32 accuracy to ~1.5e-6 abs on the
    pre-scan activations (measured: 3 spike flips in 16.7M) at bf16 PE
    rate -- 3x cheaper than native fp32 (4 passes).
  - bt = t*64+b is the matmul moving dimension, processed in blocks
    (7x512 + 2x256 bt). Loops run ko-OUTER so weights and x stream
    ko-synchronized: the PE starts after ~1MB of DMA instead of ~6MB.
    The 4 psum banks (one per 128-feature tile) accumulate in parallel
    per block; block results pipeline psum copy (ACT) -> scan (DVE) ->
    spike DMA under the next block's matmuls.
  - The kernel emits "no-spike" indicators (mem > thr); the host
    computes spikes = 1 - nspk (exact complement of float 0/1).
"""

import os
import sys

for _p in ("/opt/trn_rl_repo", "/root/.axon_site", "/root/.axon_site/_ro/trn_rl_repo",
           "/root/.axon_site/_ro/pypackages"):
    if os.path.isdir(_p) and _p not in sys.path:
        sys.path.append(_p)

from contextlib import ExitStack

import ml_dtypes
import numpy as np

import concourse.bass as bass  # noqa: F401  (registers engine classes)
import concourse.mybir as mybir
import concourse.tile as tile
from concourse import bacc
from concourse.bass_utils import run_bass_kernel_spmd

# Problem constants
B, F_IN, F_OUT, T = 64, 4096, 4096, 64
KVCO = 5.0e8
KPD = 1.0 / np.pi
TIMESTEP = 1e-9
SCALING = KVCO * KPD * TIMESTEP / 2.0
THRESHOLD = -0.5

NCORES = 8
FS = F_OUT // NCORES      # 512 out-features per core
P = 128                   # partitions
M = FS // P               # 4 feature tiles per core
KO = F_IN // P            # 32 contraction tiles
KG = 2                    # ko per x/w DMA chunk
BT = B * T                # 4096 moving dim (t-major: bt = t*B + b)
BLOCKS = [512] * 7 + [256, 128, 128]   # bt per block (small tail blocks)
assert sum(BLOCKS) == BT
XCOLS = KO * BT           # per-partition x elements

_dt = mybir.dt

_CACHE = {}


def _build_program():
    """Emit the SPMD Tile program (identical on all 8 cores)."""
    nc = bacc.Bacc("TRN2", target_bir_lowering=False, debug=False)

    xhi_d = nc.dram_tensor("xhi", [P, XCOLS], _dt.bfloat16, kind="ExternalInput").ap()
    xlo_d = nc.dram_tensor("xlo", [P, XCOLS], _dt.bfloat16, kind="ExternalInput").ap()
    whi_d = nc.dram_tensor("whi", [P, KO * M * P], _dt.bfloat16, kind="ExternalInput").ap()
    wlo_d = nc.dram_tensor("wlo", [P, KO * M * P], _dt.bfloat16, kind="ExternalInput").ap()
    mlt_d = nc.dram_tensor("multf", [P, M * B], _dt.float32, kind="ExternalInput").ap()
    out_d = nc.dram_tensor("nspk", [P, M * BT], _dt.float32, kind="ExternalOutput").ap()
    out3 = out_d.rearrange("p (m bt) -> p m bt", m=M)
    whi4 = whi_d.rearrange("p (k m c) -> p k m c", k=KO, m=M)
    wlo4 = wlo_d.rearrange("p (k m c) -> p k m c", k=KO, m=M)

    with tile.TileContext(nc) as tc, ExitStack() as ctx:
        wpool = ctx.enter_context(tc.tile_pool(name="wpool", bufs=1))
        cpool = ctx.enter_context(tc.tile_pool(name="cpool", bufs=1))
        xpool = ctx.enter_context(tc.tile_pool(name="xpool", bufs=8))
        ypool = ctx.enter_context(tc.tile_pool(name="ypool", bufs=2))
        npool = ctx.enter_context(tc.tile_pool(name="npool", bufs=2))
        spool = ctx.enter_context(tc.tile_pool(name="spool", bufs=1))
        tpool = ctx.enter_context(tc.tile_pool(name="tpool", bufs=2))
        pspool = ctx.enter_context(tc.tile_pool(name="pspool", bufs=8, space="PSUM"))

        w_hi = wpool.tile([P, KO, M, P], _dt.bfloat16, tag="whi", name="w_hi")
        w_lo = wpool.tile([P, KO, M, P], _dt.bfloat16, tag="wlo", name="w_lo")
        mlt = cpool.tile([P, M, B], _dt.float32, tag="mlt", name="mlt")
        mem = spool.tile([P, M, B], _dt.float32, tag="mem", name="mem")
        nc.vector.memset(mem[:], 0.0)

        base = 0
        for nb, NJ in enumerate(BLOCKS):
            TL = NJ // B
            y = ypool.tile([P, M, TL, B], _dt.float32, tag="y", name="y")
            ps = [pspool.tile([P, TL, B], _dt.float32, tag="ps", name=f"ps{m}")
                  for m in range(M)]
            for kg in range(KO // KG):
                if nb == 0:
                    # stream weights ko-synchronized with x on both queues
                    nc.sync.dma_start(w_hi[:, kg * KG:(kg + 1) * KG], whi4[:, kg * KG:(kg + 1) * KG])
                    nc.gpsimd.dma_start(w_lo[:, kg * KG:(kg + 1) * KG], wlo4[:, kg * KG:(kg + 1) * KG])
                xh = xpool.tile([P, KG, NJ], _dt.bfloat16, tag="xh", name="xh")
                xl = xpool.tile([P, KG, NJ], _dt.bfloat16, tag="xl", name="xl")
                xhsrc = xhi_d[:, base + kg * KG * NJ: base + (kg + 1) * KG * NJ] \
                    .rearrange("p (k j) -> p k j", k=KG)
                xlsrc = xlo_d[:, base + kg * KG * NJ: base + (kg + 1) * KG * NJ] \
                    .rearrange("p (k j) -> p k j", k=KG)
                if nb == 0 and kg == 0:
                    # halve the first chunks so the PE starts sooner
                    nc.sync.dma_start(xh[:, 0:1, :], xhsrc[:, 0:1, :])
                    nc.gpsimd.dma_start(xl[:, 0:1, :], xlsrc[:, 0:1, :])
                    nc.sync.dma_start(xh[:, 1:, :], xhsrc[:, 1:, :])
                    nc.gpsimd.dma_start(xl[:, 1:, :], xlsrc[:, 1:, :])
                else:
                    nc.sync.dma_start(xh[:], xhsrc)
                    nc.gpsimd.dma_start(xl[:], xlsrc)
                if nb == 0 and kg == 1:
                    nc.gpsimd.dma_start(mlt[:], mlt_d.rearrange("p (m b) -> p m b", m=M))
                for kk in range(KG):
                    ko = kg * KG + kk
                    for m in range(M):
                        nc.tensor.matmul(ps[m][:], w_hi[:, ko, m, :], xh[:, kk, :],
                                         start=(ko == 0), stop=False)
                        nc.tensor.matmul(ps[m][:], w_hi[:, ko, m, :], xl[:, kk, :],
                                         start=False, stop=False)
                        nc.tensor.matmul(ps[m][:], w_lo[:, ko, m, :], xh[:, kk, :],
                                         start=False, stop=(ko == KO - 1))
            for m in range(M):
                nc.scalar.copy(y[:, m, :, :], ps[m][:])

            nsp = npool.tile([P, M, TL, B], _dt.float32, tag="nsp", name="nsp")
            for tl in range(TL):
                tmp = tpool.tile([P, M, B], _dt.float32, tag="tmp", name="tmp")
                nc.vector.tensor_mul(tmp[:], mem[:], mlt[:])
                nc.vector.tensor_add(mem[:], tmp[:], y[:, :, tl, :])
                nc.vector.tensor_scalar(nsp[:, :, tl, :], mem[:], THRESHOLD, None,
                                        mybir.AluOpType.is_gt)
                nc.vector.tensor_mul(mem[:], mem[:], nsp[:, :, tl, :])

            # last blocks' outputs go via HWDGE: the SWDGE tail drain is slow
            out_eng = nc.sync if nb >= len(BLOCKS) - 2 else nc.gpsimd
            out_eng.dma_start(out3[:, :, base // KO: base // KO + NJ],
                              nsp[:].rearrange("p m tl b -> p m (tl b)"))
            base += KO * NJ

    nc.compile()
    return nc


def _prep_inputs(inputs: np.ndarray, W: np.ndarray):
    """Host-side preprocessing: neuron constants, scale folding, hi/lo
    bf16 split, and per-core data layouts."""
    f32 = np.float32
    rs = W.sum(axis=1, dtype=np.float64).astype(f32)
    ncst = np.maximum((f32(1.0) + rs) / f32(SCALING), f32(0.0)).astype(f32)
    scale = (f32(-1.0) / (f32(1.0) + ncst)).astype(f32)
    mult = ((ncst - f32(1.0)) / (ncst + f32(1.0))).astype(f32)

    Ws = (W * scale[:, None]).astype(f32)          # fold scale into W rows
    wT = np.ascontiguousarray(Ws.T)                # [F_IN, F_OUT]
    wT_hi = wT.astype(ml_dtypes.bfloat16)
    wT_lo = (wT - wT_hi.astype(f32)).astype(ml_dtypes.bfloat16)

    # x_flat[i, t*B + b] = inputs[b, i, t]
    x_flat = np.ascontiguousarray(inputs.transpose(1, 2, 0)).reshape(F_IN, BT)
    x_hi = x_flat.astype(ml_dtypes.bfloat16)
    x_lo = (x_flat - x_hi.astype(f32)).astype(ml_dtypes.bfloat16)

    def xlayout(xa):
        # [F_IN, BT] -> [P, KO*BT], per block: [p][ko][j] flattened
        parts = []
        cum = 0
        for NJ in BLOCKS:
            seg = xa[:, cum:cum + NJ]              # [F_IN, NJ]
            parts.append(np.ascontiguousarray(
                seg.reshape(KO, P, NJ).transpose(1, 0, 2)).reshape(P, KO * NJ))
            cum += NJ
        return np.ascontiguousarray(np.concatenate(parts, axis=1))

    xh_b = xlayout(x_hi)
    xl_b = xlayout(x_lo)

    in_maps = []
    for c in range(NCORES):
        sl = slice(c * FS, (c + 1) * FS)

        def wlayout(wa):
            # [F_IN, FS] -> [P, KO*M*P]: w[p, ko, m, cc] = wT[ko*P+p, m*P+cc]
            return np.ascontiguousarray(
                wa[:, sl].reshape(KO, P, M, P).transpose(1, 0, 2, 3)).reshape(P, KO * M * P)

        mc = mult[sl].reshape(M, P).T              # [P, M]
        multf = np.ascontiguousarray(
            np.broadcast_to(mc[:, :, None], (P, M, B))).reshape(P, M * B).astype(f32)

        in_maps.append({
            "xhi": xh_b, "xlo": xl_b,
            "whi": wlayout(wT_hi), "wlo": wlayout(wT_lo),
            "multf": multf,
        })
    return in_maps


def kernel(inputs: np.ndarray, W: np.ndarray) -> np.ndarray:
    if "nc" not in _CACHE:
        _CACHE["nc"] = _build_program()
    nc = _CACHE["nc"]

    in_maps = _prep_inputs(np.asarray(inputs, np.float32), np.asarray(W, np.float32))

    kw = {}
    if os.environ.get("KERNEL_TRACE"):
        kw = {"trace": True}
    res = run_bass_kernel_spmd(nc, in_maps, core_ids=list(range(NCORES)), **kw)
    _CACHE["last_result"] = res
    if res.exec_time_ns is not None:
        print(f"HW exec time: {res.exec_time_ns} ns")

    full = np.empty((B, F_OUT, T), np.float32)
    for c in range(NCORES):
        a = res.results[c]["nspk"].reshape(P, M, T, B)
        a = a.transpose(3, 1, 0, 2)                # [b, m, p, t]
        full[:, c * FS:(c + 1) * FS, :] = a.reshape(B, FS, T)
    return (np.float32(1.0) - full).astype(np.float32)


# revision 14
# speedup vs baseline: 1.0599x; 1.0007x over previous
"""Trainium2 Bass kernel for nn_Dense_25202868092933.

Computes: outs = einsum('bit,oi->bot', inputs, W); then a 64-step
leaky-integrate-and-fire neuron scan with hard reset:
    mem = mult*mem + scale*outs[..., t];  spk = (mem <= -0.5);  mem *= (1-spk)

Strategy (8 NeuronCores, no cross-core communication):
  - Tensor-parallel over out_features: each core owns 512 rows of W,
    the scale/mult vectors for those rows, and the scan state.
  - scale is folded into W on the host (W' = diag(scale) @ W), so the
    matmul directly produces scale*outs.
  - The matmul runs as a 3-pass bf16 hi/lo split (x_hi@W_hi + x_lo@W_hi
    + x_hi@W_lo), which reproduces fp32 accuracy to ~1.5e-6 abs on the
    pre-scan activations (measured: 3 spike flips in 16.7M) at bf16 PE
    rate -- 3x cheaper than native fp32 (4 passes).
  - bt = t*64+b is the matmul moving dimension, processed in blocks
    (7x512 + 2x256 bt). Loops run ko-OUTER so weights and x stream
    ko-synchronized: the PE starts after ~1MB of DMA instead of ~6MB.
    The 4 psum banks (one per 128-feature tile) accumulate in parallel
    per block; block results pipeline psum copy (ACT) -> scan (DVE) ->
    spike DMA under the next block's matmuls.
  - The kernel emits "no-spike" indicators (mem > thr); the host
    computes spikes = 1 - nspk (exact complement of float 0/1).
"""

import os
import sys

for _p in ("/opt/trn_rl_repo", "/root/.axon_site", "/root/.axon_site/_ro/trn_rl_repo",
           "/root/.axon_site/_ro/pypackages"):
    if os.path.isdir(_p) and _p not in sys.path:
        sys.path.append(_p)

from contextlib import ExitStack

import ml_dtypes
import numpy as np

import concourse.bass as bass  # noqa: F401  (registers engine classes)
import concourse.mybir as mybir
import concourse.tile as tile
from concourse import bacc
from concourse.bass_utils import run_bass_kernel_spmd

# Problem constants
B, F_IN, F_OUT, T = 64, 4096, 4096, 64
KVCO = 5.0e8
KPD = 1.0 / np.pi
TIMESTEP = 1e-9
SCALING = KVCO * KPD * TIMESTEP / 2.0
THRESHOLD = -0.5

NCORES = 8
FS = F_OUT // NCORES      # 512 out-features per core
P = 128                   # partitions
M = FS // P               # 4 feature tiles per core
KO = F_IN // P            # 32 contraction tiles
KG = 2                    # ko per x/w DMA chunk
BT = B * T                # 4096 moving dim (t-major: bt = t*B + b)
BLOCKS = [512] * 7 + [256, 128, 128]   # bt per block (small tail blocks)
assert sum(BLOCKS) == BT
XCOLS = KO * BT           # per-partition x elements

_dt = mybir.dt

_CACHE = {}


def _build_program():
    """Emit the SPMD Tile program (identical on all 8 cores)."""
    nc = bacc.Bacc("TRN2", target_bir_lowering=False, debug=False)

    xhi_d = nc.dram_tensor("xhi", [P, XCOLS], _dt.bfloat16, kind="ExternalInput").ap()
    xlo_d = nc.dram_tensor("xlo", [P, XCOLS], _dt.bfloat16, kind="ExternalInput").ap()
    whi_d = nc.dram_tensor("whi", [P, KO * M * P], _dt.bfloat16, kind="ExternalInput").ap()
    wlo_d = nc.dram_tensor("wlo", [P, KO * M * P], _dt.bfloat16, kind="ExternalInput").ap()
    mlt_d = nc.dram_tensor("multf", [P, M * B], _dt.float32, kind="ExternalInput").ap()
    out_d = nc.dram_tensor("nspk", [P, M * BT], _dt.float32, kind="ExternalOutput").ap()
    out3 = out_d.rearrange("p (m bt) -> p m bt", m=M)
    whi4 = whi_d.rearrange("p (k m c) -> p k m c", k=KO, m=M)
    wlo4 = wlo_d.rearrange("p (k m c) -> p k m c", k=KO, m=M)

    with tile.TileContext(nc) as tc, ExitStack() as ctx:
        wpool = ctx.enter_context(tc.tile_pool(name="wpool", bufs=1))
        cpool = ctx.enter_context(tc.tile_pool(name="cpool", bufs=1))
        xpool = ctx.enter_context(tc.tile_pool(name="xpool", bufs=8))
        ypool = ctx.enter_context(tc.tile_pool(name="ypool", bufs=2))
        npool = ctx.enter_context(tc.tile_pool(name="npool", bufs=2))
        spool = ctx.enter_context(tc.tile_pool(name="spool", bufs=1))
        tpool = ctx.enter_context(tc.tile_pool(name="tpool", bufs=2))
        pspool = ctx.enter_context(tc.tile_pool(name="pspool", bufs=8, space="PSUM"))

        w_hi = wpool.tile([P, KO, M, P], _dt.bfloat16, tag="whi", name="w_hi")
        w_lo = wpool.tile([P, KO, M, P], _dt.bfloat16, tag="wlo", name="w_lo")
        mlt = cpool.tile([P, M, B], _dt.float32, tag="mlt", name="mlt")
        mem = spool.tile([P, M, B], _dt.float32, tag="mem", name="mem")
        nc.vector.memset(mem[:], 0.0)

        # PE warmup: ~10 throwaway matmuls on scratch tiles run during the
        # initial DMA ramp (PE otherwise idle ~5us), so the HAM clock gate
        # is already at 8/8 (2.4 GHz) when the first real matmul issues.
        wm_l = cpool.tile([P, P], _dt.bfloat16, tag="wm_l", name="wm_l")
        wm_r = cpool.tile([P, 512], _dt.bfloat16, tag="wm_r", name="wm_r")
        nc.vector.memset(wm_l[:], 0.0)
        nc.vector.memset(wm_r[:], 0.0)
        psw = pspool.tile([P, 512], _dt.float32, tag="ps", name="psw")
        for i in range(10):
            nc.tensor.matmul(psw[:], wm_l[:], wm_r[:], start=(i == 0), stop=(i == 9))

        base = 0
        for nb, NJ in enumerate(BLOCKS):
            TL = NJ // B
            y = ypool.tile([P, M, TL, B], _dt.float32, tag="y", name="y")
            ps = [pspool.tile([P, TL, B], _dt.float32, tag="ps", name=f"ps{m}")
                  for m in range(M)]
            for kg in range(KO // KG):
                if nb == 0:
                    # stream weights ko-synchronized with x on both queues;
                    # halve the very first chunk for lower first-MM latency
                    if kg == 0:
                        nc.sync.dma_start(w_hi[:, 0:1], whi4[:, 0:1])
                        nc.gpsimd.dma_start(w_lo[:, 0:1], wlo4[:, 0:1])
                        nc.sync.dma_start(w_hi[:, 1:2], whi4[:, 1:2])
                        nc.gpsimd.dma_start(w_lo[:, 1:2], wlo4[:, 1:2])
                    else:
                        nc.sync.dma_start(w_hi[:, kg * KG:(kg + 1) * KG], whi4[:, kg * KG:(kg + 1) * KG])
                        nc.gpsimd.dma_start(w_lo[:, kg * KG:(kg + 1) * KG], wlo4[:, kg * KG:(kg + 1) * KG])
                xh = xpool.tile([P, KG, NJ], _dt.bfloat16, tag="xh", name="xh")
                xl = xpool.tile([P, KG, NJ], _dt.bfloat16, tag="xl", name="xl")
                xhsrc = xhi_d[:, base + kg * KG * NJ: base + (kg + 1) * KG * NJ] \
                    .rearrange("p (k j) -> p k j", k=KG)
                xlsrc = xlo_d[:, base + kg * KG * NJ: base + (kg + 1) * KG * NJ] \
                    .rearrange("p (k j) -> p k j", k=KG)
                if nb == 0 and kg == 0:
                    # halve the first chunks so the PE starts sooner
                    nc.sync.dma_start(xh[:, 0:1, :], xhsrc[:, 0:1, :])
                    nc.gpsimd.dma_start(xl[:, 0:1, :], xlsrc[:, 0:1, :])
                    nc.sync.dma_start(xh[:, 1:, :], xhsrc[:, 1:, :])
                    nc.gpsimd.dma_start(xl[:, 1:, :], xlsrc[:, 1:, :])
                else:
                    nc.sync.dma_start(xh[:], xhsrc)
                    nc.gpsimd.dma_start(xl[:], xlsrc)
                if nb == 0 and kg == 1:
                    nc.gpsimd.dma_start(mlt[:], mlt_d.rearrange("p (m b) -> p m b", m=M))
                for kk in range(KG):
                    ko = kg * KG + kk
                    for m in range(M):
                        nc.tensor.matmul(ps[m][:], w_hi[:, ko, m, :], xh[:, kk, :],
                                         start=(ko == 0), stop=False)
                        nc.tensor.matmul(ps[m][:], w_hi[:, ko, m, :], xl[:, kk, :],
                                         start=False, stop=False)
                        nc.tensor.matmul(ps[m][:], w_lo[:, ko, m, :], xh[:, kk, :],
                                         start=False, stop=(ko == KO - 1))
            for m in range(M):
                nc.scalar.copy(y[:, m, :, :], ps[m][:])

            nsp = npool.tile([P, M, TL, B], _dt.float32, tag="nsp", name="nsp")
            for tl in range(TL):
                tmp = tpool.tile([P, M, B], _dt.float32, tag="tmp", name="tmp")
                nc.vector.tensor_mul(tmp[:], mem[:], mlt[:])
                nc.vector.tensor_add(mem[:], tmp[:], y[:, :, tl, :])
                nc.vector.tensor_scalar(nsp[:, :, tl, :], mem[:], THRESHOLD, None,
                                        mybir.AluOpType.is_gt)
                nc.vector.tensor_mul(mem[:], mem[:], nsp[:, :, tl, :])

            # last blocks' outputs go via HWDGE: the SWDGE tail drain is slow
            out_eng = nc.sync if nb >= len(BLOCKS) - 2 else nc.gpsimd
            out_eng.dma_start(out3[:, :, base // KO: base // KO + NJ],
                              nsp[:].rearrange("p m tl b -> p m (tl b)"))
            base += KO * NJ

    nc.compile()
    return nc


def _prep_inputs(inputs: np.ndarray, W: np.ndarray):
    """Host-side preprocessing: neuron constants, scale folding, hi/lo
    bf16 split, and per-core data layouts."""
    f32 = np.float32
    rs = W.sum(axis=1, dtype=np.float64).astype(f32)
    ncst = np.maximum((f32(1.0) + rs) / f32(SCALING), f32(0.0)).astype(f32)
    scale = (f32(-1.0) / (f32(1.0) + ncst)).astype(f32)
    mult = ((ncst - f32(1.0)) / (ncst + f32(1.0))).astype(f32)

    Ws = (W * scale[:, None]).astype(f32)          # fold scale into W rows
    wT = np.ascontiguousarray(Ws.T)                # [F_IN, F_OUT]
    wT_hi = wT.astype(ml_dtypes.bfloat16)
    wT_lo = (wT - wT_hi.astype(f32)).astype(ml_dtypes.bfloat16)

    # x_flat[i, t*B + b] = inputs[b, i, t]
    x_flat = np.ascontiguousarray(inputs.transpose(1, 2, 0)).reshape(F_IN, BT)
    x_hi = x_flat.astype(ml_dtypes.bfloat16)
    x_lo = (x_flat - x_hi.astype(f32)).astype(ml_dtypes.bfloat16)

    def xlayout(xa):
        # [F_IN, BT] -> [P, KO*BT], per block: [p][ko][j] flattened
        parts = []
        cum = 0
        for NJ in BLOCKS:
            seg = xa[:, cum:cum + NJ]              # [F_IN, NJ]
            parts.append(np.ascontiguousarray(
                seg.reshape(KO, P, NJ).transpose(1, 0, 2)).reshape(P, KO * NJ))
            cum += NJ
        return np.ascontiguousarray(np.concatenate(parts, axis=1))

    xh_b = xlayout(x_hi)
    xl_b = xlayout(x_lo)

    in_maps = []
    for c in range(NCORES):
        sl = slice(c * FS, (c + 1) * FS)

        def wlayout(wa):
            # [F_IN, FS] -> [P, KO*M*P]: w[p, ko, m, cc] = wT[ko*P+p, m*P+cc]
            return np.ascontiguousarray(
                wa[:, sl].reshape(KO, P, M, P).transpose(1, 0, 2, 3)).reshape(P, KO * M * P)

        mc = mult[sl].reshape(M, P).T              # [P, M]
        multf = np.ascontiguousarray(
            np.broadcast_to(mc[:, :, None], (P, M, B))).reshape(P, M * B).astype(f32)

        in_maps.append({
            "xhi": xh_b, "xlo": xl_b,
            "whi": wlayout(wT_hi), "wlo": wlayout(wT_lo),
            "multf": multf,
        })
    return in_maps


def kernel(inputs: np.ndarray, W: np.ndarray) -> np.ndarray:
    if "nc" not in _CACHE:
        _CACHE["nc"] = _build_program()
    nc = _CACHE["nc"]

    in_maps = _prep_inputs(np.asarray(inputs, np.float32), np.asarray(W, np.float32))

    kw = {}
    if os.environ.get("KERNEL_TRACE"):
        kw = {"trace": True}
    res = run_bass_kernel_spmd(nc, in_maps, core_ids=list(range(NCORES)), **kw)
    _CACHE["last_result"] = res
    if res.exec_time_ns is not None:
        print(f"HW exec time: {res.exec_time_ns} ns")

    full = np.empty((B, F_OUT, T), np.float32)
    for c in range(NCORES):
        a = res.results[c]["nspk"].reshape(P, M, T, B)
        a = a.transpose(3, 1, 0, 2)                # [b, m, p, t]
        full[:, c * FS:(c + 1) * FS, :] = a.reshape(B, FS, T)
    return (np.float32(1.0) - full).astype(np.float32)
